# revision 1
# baseline (speedup 1.0000x reference)
"""AWPINN wavelet-PINN kernel for 8x Trainium2 NeuronCores (Bass/Tile).

Math: for each point i and wavelet k (N=65536, K=512):
  xt = wx*x - bx (same y,z);  s = xt^2+yt^2+zt^2;  E = exp(-0.5*s)
  W  = xt*yt*zt*E          (reference's xw*yw*zw = -W)
  output = sum_k (-coeff*scale)_k * W + bias
  d2u_dx2 = sum_k (coeff*scale*wx^2)_k * (3 - xt^2) * W   (same y,z)

Device structure:
  - s and T3=xt*yt*zt are low-rank bilinear forms in per-point features
    F = [x2,y2,z2,xyz,xy,xz,yz,x,y,z,1] -> TensorEngine matmuls
    (contraction = features, M = 128 wavelets/block, FD = 512 points).
  - All matmuls fp16 (1 cycle/column on PE; fp32/fp32r are ~3x slower).
    Near-fp32 precision via hi/lo splits stacked along the contraction dim:
    [Lh;Ll;Lh] @ [Fh;Fh;Fl] == L @ F with ~2^-21 products. The constant
    feature's lo-row is dropped -> exactly 32 rows, so the four feature
    matmuls of a k-block pair rotate over the PE's four 32-row groups
    (tile_position) and their weight loads overlap the previous matmul.
  - d2 terms decompose via xt^2 = wx^2*x^2 - 2*wx*bx*x + bx^2 into 3
    matvec columns each -> one [128k, 10] output matmul (hi + lo lhsT)
    per k-block, combined per-point in a tiny [128,64]-layout epilogue.
    The hi and lo output matmuls accumulate into disjoint partition ranges
    (col groups) so consecutive matmuls overlap instead of serializing on
    the PSUM drain; ACT+DVE merge the two partial R's during the drain.
  - k-blocks are processed in pairs; exp (ACT) and W=T3*E (DVE) run at
    FD=1024 over both psum banks of a pair to amortize instruction cost.
  - R bounces through DRAM in a point-major layout so the epilogue reload
    is a plain 2D slice; the epilogue runs per point-half, overlapping the
    main loop for the first half.
Data parallel over points: each core handles 8192 points; no collectives.
"""

import numpy as np

N_TOTAL = 65536
K_TOTAL = 512
N_CORES = 8
NP_CORE = N_TOTAL // N_CORES        # 8192 points per core
CHUNK = 512                         # points per matmul (PSUM bank = 512 fp32)
N_CHUNKS = NP_CORE // CHUNK         # 16
KBLK = K_TOTAL // 128               # 4 wavelet blocks of 128
EPP = NP_CORE // 128                # 64 = free dim of [128, 64] point layout
NFEAT = 11                          # features per point
NST = 32                            # stacked contraction rows (ones-lo dropped)
LO_SPLIT = True                     # hi+lo split of output-matmul lhsT
PACK_FEATURES = True                # tile_position row-group rotation

_COMPILED = {}


def _split16(a):
    """Split fp32 into fp16 hi + fp16 lo (hi+lo carries ~21 mantissa bits)."""
    a = np.ascontiguousarray(a, np.float32)
    hi = a.astype(np.float16)
    lo = np.float32(a - hi.astype(np.float32)).astype(np.float16)
    return hi, lo


def _stack32(L):
    """[11,n] fp32 coeffs -> [32,n] fp16 stack [Lh; Ll; Lh[:10]]."""
    Lh, Ll = _split16(L)
    return np.concatenate([Lh, Ll, Lh[:NFEAT - 1]], axis=0)


def _build_program():
    import concourse.bacc as bacc
    import concourse.mybir as mybir
    import concourse.tile as tile

    f32 = mybir.dt.float32
    f16 = mybir.dt.float16
    AF = mybir.ActivationFunctionType

    nc = bacc.Bacc("TRN2", target_bir_lowering=False, debug=False)

    # fst: feature stack; replicated on-device at partition offsets 0/32/64/96
    fst_d = nc.dram_tensor("fst", [NST, NP_CORE], f16, kind="ExternalInput")
    # lst: rows 0-31 Ls-stack, 32-63 Lt-stack, 64-95 Ls, 96-127 Lt;
    # columns grouped by k-block
    lst_d = nc.dram_tensor("lst", [128, K_TOTAL], f16, kind="ExternalInput")
    loh_d = nc.dram_tensor("loh", [128, KBLK * 10], f16, kind="ExternalInput")
    lol_d = nc.dram_tensor("lol", [128, KBLK * 10], f16, kind="ExternalInput")
    ep_d = nc.dram_tensor("ep", [6, NP_CORE], f32, kind="ExternalInput")
    out_d = nc.dram_tensor("out", [4, NP_CORE], f32, kind="ExternalOutput")

    with tile.TileContext(nc) as tc:
        with (
            tc.tile_pool(name="persist", bufs=1) as pp,
            tc.tile_pool(name="work", bufs=4) as wp,
            tc.tile_pool(name="psum_s", bufs=2, space="PSUM") as psps,
            tc.tile_pool(name="psum_t", bufs=2, space="PSUM") as pspt,
            tc.tile_pool(name="psum_out", bufs=2, space="PSUM") as pso,
            tc.tile_pool(name="wpool", bufs=8) as wpool,
            tc.tile_pool(name="dram", bufs=1, space="DRAM") as dp,
        ):
            lst_t = pp.tile([128, K_TOTAL], f16, tag="lst")
            nc.gpsimd.dma_start(lst_t[:], lst_d[:])

            # persistent feature stack, replicated across the 4 row groups;
            # loaded in point-eighths so compute starts after the first slice
            f_all = pp.tile([4 * NST, NP_CORE], f16, tag="f_all")
            # first two slices are single chunks so matmuls start sooner
            bounds = [0, CHUNK, 2 * CHUNK] + [
                q * NP_CORE // 8 for q in range(2, 9)]
            for q in range(len(bounds) - 1):
                qs = slice(bounds[q], bounds[q + 1])
                for g in range(4):
                    eng = nc.scalar if (q == 0 and g >= 2) else nc.sync
                    eng.dma_start(f_all[32 * g:32 * (g + 1), qs],
                                  fst_d[:, qs])
                if q == 0:
                    loh_t = pp.tile([128, KBLK * 10], f16, tag="loh")
                    nc.scalar.dma_start(loh_t[:], loh_d[:])
                    if LO_SPLIT:
                        lol_t = pp.tile([128, KBLK * 10], f16, tag="lol")
                        nc.scalar.dma_start(lol_t[:], lol_d[:])

            r_rows = pp.tile([10, NP_CORE], f32, tag="r_rows")
            # R in DRAM, point-major: r_dram[p, r*64 + f] for point p*64+f
            r_dram = dp.tile([128, 10 * EPP], f32, tag="r_dram")

            ep_t = []
            for i in range(6):  # x2, x, y2, y, z2, z
                t = pp.tile([128, EPP], f32, tag=f"ep{i}")
                nc.sync.dma_start(
                    t[:], ep_d[i:i + 1, :].rearrange("o (p f) -> (o p) f", p=128))
                ep_t.append(t)

            pending_drain = []

            def emit_drain():
                if not pending_drain:
                    return
                dc, dpo_a, dpo_b = pending_drain.pop()
                dst = r_rows[:, dc * CHUNK:(dc + 1) * CHUNK]
                nc.scalar.copy(dst, dpo_a)
                if LO_SPLIT:
                    nc.vector.scalar_tensor_tensor(
                        dst, dpo_b, 1.0, dst,
                        mybir.AluOpType.mult, mybir.AluOpType.add)
                nc.sync.dma_start(
                    r_dram[dc * 8:(dc + 1) * 8, :].rearrange(
                        "p (r f) -> r p f", r=10),
                    dst.rearrange("r (p f) -> r p f", p=8))

            for c in range(N_CHUNKS):
                f_t = f_all[:, c * CHUNK:(c + 1) * CHUNK]
                po_ab = pso.tile([42, CHUNK], f32, tag="po_ab")
                po_a, po_b = po_ab[0:10, :], po_ab[32:42, :]
                w_ts = []
                for p in range(KBLK // 2):      # k-block pairs
                    kb0, kb1 = 2 * p, 2 * p + 1
                    # s halves in separate single-bank tiles so each EXP
                    # half releases its bank for the next pair's s-matmul
                    # sooner (shortens the EXP->matmul->EXP chain)
                    ps = [psps.tile([128, CHUNK], f32, tag="ps_s",
                                    name=f"ps{c}_{p}_{i}") for i in range(2)]
                    ps_t = pspt.tile([128, 2 * CHUNK], f32, tag="ps_t")
                    # 4 feature matmuls burst over the four 32-row groups
                    for g, (dst, kb) in [
                            (0, (ps[0][:], kb0)), (2, (ps[1][:], kb1)),
                            (1, (ps_t[:, 0:CHUNK], kb0)),
                            (3, (ps_t[:, CHUNK:], kb1))]:
                        nc.tensor.matmul(
                            dst,
                            lst_t[32 * g:32 * (g + 1), kb * 128:(kb + 1) * 128],
                            f_t[32 * g:32 * (g + 1), :],
                            start=True, stop=True,
                            tile_position=(32 * g, 0) if PACK_FEATURES else None)
                    e_t = wp.tile([128, 2 * CHUNK], f32, tag="e")
                    nc.scalar.activation(
                        e_t[:, 0:CHUNK], ps[0][:], AF.Exp, scale=-0.5)
                    nc.scalar.activation(
                        e_t[:, CHUNK:], ps[1][:], AF.Exp, scale=-0.5)
                    w_t = wpool.tile([128, 2 * CHUNK], f16, tag="w")
                    nc.vector.tensor_mul(
                        w_t[:, 0:CHUNK], ps_t[:, 0:CHUNK], e_t[:, 0:CHUNK])
                    nc.vector.tensor_mul(
                        w_t[:, CHUNK:], ps_t[:, CHUNK:], e_t[:, CHUNK:])
                    w_ts.append(w_t)
                    if p == 0:
                        emit_drain()   # previous chunk's R drain, off the
                                       # critical path of this chunk's E/W
                # output matmuls accumulate over all four k-blocks; hi and lo
                # lhsT target alternating PSUM banks so consecutive matmuls
                # overlap (same-bank accumulation serializes on the drain)
                for kb in range(KBLK):
                    w_t = w_ts[kb // 2]
                    half = slice((kb & 1) * CHUNK, ((kb & 1) + 1) * CHUNK)
                    nc.tensor.matmul(
                        po_a, loh_t[:, kb * 10:(kb + 1) * 10], w_t[:, half],
                        start=(kb == 0), stop=(kb == KBLK - 1))
                    if LO_SPLIT:
                        nc.tensor.matmul(
                            po_b, lol_t[:, kb * 10:(kb + 1) * 10],
                            w_t[:, half],
                            start=(kb == 0), stop=(kb == KBLK - 1))
                # defer this chunk's R drain so it does not delay the next
                # chunk's E/W in the ACT/DVE queues
                pending_drain.append((c, po_a, po_b))

                # two-phase epilogue: once half the points are in r_dram,
                # rebuild d2 outputs for those points. Point i lives at
                # [i // EPP, i % EPP] in the [128, EPP] layout, so a
                # point-half is a partition-half.
                if c not in (N_CHUNKS // 2 - 1, N_CHUNKS - 1):
                    continue
                emit_drain()
                hlf = slice(0, 64) if c == N_CHUNKS // 2 - 1 else slice(64, 128)
                psl = slice((hlf.start // 64) * NP_CORE // 2,
                            (hlf.start // 64 + 1) * NP_CORE // 2)
                nc.sync.dma_start(
                    out_d[0:1, psl].rearrange("o (p f) -> (o p) f", p=64),
                    r_dram[hlf, 0:EPP])
                # R1..R9 for this half, split across two DMA queues
                rb = wp.tile([128, 9 * EPP], f32, tag="rb")
                h0 = slice(hlf.start, hlf.start + 32)
                h1 = slice(hlf.start + 32, hlf.stop)
                nc.sync.dma_start(rb[h0, :], r_dram[h0, EPP:10 * EPP])
                nc.sync.dma_start(rb[h1, :], r_dram[h1, EPP:10 * EPP])
                # first half runs mid-loop on the idle GPSIMD; the final
                # half runs in the tail where DVE is idle and 2x faster
                ee = nc.gpsimd if hlf.start == 0 else nc.vector
                for j in range(3):  # d2x, d2y, d2z
                    def rbs(idx):
                        return rb[hlf, (idx - 1) * EPP:idx * EPP]
                    sq_t, lin_t = ep_t[2 * j], ep_t[2 * j + 1]
                    m1 = wp.tile([128, EPP], f32, tag="m1")
                    ee.tensor_mul(m1[hlf, :], sq_t[hlf, :], rbs(3 * j + 1))
                    m2 = wp.tile([128, EPP], f32, tag="m2")
                    ee.tensor_mul(m2[hlf, :], lin_t[hlf, :], rbs(3 * j + 2))
                    a1 = wp.tile([128, EPP], f32, tag="a1")
                    ee.tensor_add(a1[hlf, :], m1[hlf, :], m2[hlf, :])
                    d2 = wp.tile([128, EPP], f32, tag="d2")
                    ee.tensor_add(d2[hlf, :], a1[hlf, :], rbs(3 * j + 3))
                    nc.sync.dma_start(
                        out_d[j + 1:j + 2, psl].rearrange(
                            "o (p f) -> (o p) f", p=64),
                        d2[hlf, :])
    nc.compile()
    return nc


def _get_program():
    if "nc" not in _COMPILED:
        _COMPILED["nc"] = _build_program()
    return _COMPILED["nc"]


def _host_prep(x, y, z, wx, bx, wy, by, wz, bz, coeff):
    """Build per-core input maps (features + coefficient matrices)."""
    f8 = np.float64
    wx64, bx64 = wx.astype(f8), bx.astype(f8)
    wy64, by64 = wy.astype(f8), by.astype(f8)
    wz64, bz64 = wz.astype(f8), bz.astype(f8)
    c64 = coeff.astype(f8)
    sc = np.sqrt(np.clip(wx64 * wy64 * wz64, 1e-12, None))
    Z = np.zeros_like(wx64)

    # s = xt^2 + yt^2 + zt^2 over features [x2,y2,z2,xyz,xy,xz,yz,x,y,z,1]
    Ls = np.stack([
        wx64 ** 2, wy64 ** 2, wz64 ** 2, Z, Z, Z, Z,
        -2 * wx64 * bx64, -2 * wy64 * by64, -2 * wz64 * bz64,
        bx64 ** 2 + by64 ** 2 + bz64 ** 2,
    ]).astype(np.float32)                      # [11, K]
    # T3 = xt*yt*zt
    Lt = np.stack([
        Z, Z, Z,
        wx64 * wy64 * wz64, -wx64 * wy64 * bz64, -wx64 * by64 * wz64,
        -bx64 * wy64 * wz64, wx64 * by64 * bz64, bx64 * wy64 * bz64,
        bx64 * by64 * wz64, -bx64 * by64 * bz64,
    ]).astype(np.float32)                      # [11, K]
    b1 = c64 * sc * wx64 ** 2
    b2 = c64 * sc * wy64 ** 2
    b3 = c64 * sc * wz64 ** 2
    Lo = np.stack([
        -c64 * sc,
        -b1 * wx64 ** 2, 2 * b1 * wx64 * bx64, b1 * (3 - bx64 ** 2),
        -b2 * wy64 ** 2, 2 * b2 * wy64 * by64, b2 * (3 - by64 ** 2),
        -b3 * wz64 ** 2, 2 * b3 * wz64 * bz64, b3 * (3 - bz64 ** 2),
    ], axis=1).astype(np.float32)              # [K, 10]

    Ls32 = _stack32(Ls)                        # [32, K] fp16
    Lt32 = _stack32(Lt)
    lst_pack = np.concatenate([Ls32, Lt32, Ls32, Lt32], axis=0)  # [128, K]
    Loh, Lol = _split16(Lo)
    loh_pack = np.concatenate(
        [Loh[kb * 128:(kb + 1) * 128] for kb in range(KBLK)], axis=1)  # [128, 40]
    lol_pack = np.concatenate(
        [Lol[kb * 128:(kb + 1) * 128] for kb in range(KBLK)], axis=1)

    in_maps = []
    for cid in range(N_CORES):
        sl = slice(cid * NP_CORE, (cid + 1) * NP_CORE)
        xs, ys, zs = (np.ascontiguousarray(a[sl], np.float32) for a in (x, y, z))
        F = np.stack([
            xs * xs, ys * ys, zs * zs, xs * ys * zs, xs * ys, xs * zs,
            ys * zs, xs, ys, zs, np.ones_like(xs),
        ]).astype(np.float32)                  # [11, NP_CORE]
        Fh, Fl = _split16(F)
        fst = np.concatenate([Fh, Fh, Fl[:NFEAT - 1]], axis=0)   # [32, NP]
        ep = np.stack([xs * xs, xs, ys * ys, ys, zs * zs, zs]).astype(np.float32)
        in_maps.append({
            "fst": fst, "lst": lst_pack,
            "loh": loh_pack, "lol": lol_pack, "ep": ep,
        })
    return in_maps


def _run_device(in_maps, trace=False):
    from concourse.bass_utils import run_bass_kernel_spmd
    nc = _get_program()
    last_err = None
    for _attempt in range(3):
        try:
            return run_bass_kernel_spmd(
                nc, in_maps, list(range(N_CORES)), trace=trace)
        except Exception as ex:  # transient NRT device errors recover on retry
            last_err = ex
    raise last_err


def kernel(x, y, z, wx, bx, wy, by, wz, bz, coeff, bias, _trace=False):
    x, y, z = (np.asarray(a, np.float32) for a in (x, y, z))
    in_maps = _host_prep(
        x, y, z,
        *(np.asarray(a, np.float32) for a in (wx, bx, wy, by, wz, bz, coeff)))
    res = _run_device(in_maps, trace=_trace)
    outs = [res.results[cid]["out"] for cid in range(N_CORES)]
    full = np.concatenate(outs, axis=1)        # [4, N_TOTAL]
    bias_f = np.float32(np.asarray(bias))
    output = (full[0] + bias_f).astype(np.float32)
    if _trace:
        kernel._last_results = res
    return (output, full[1].copy(), full[2].copy(), full[3].copy())



# revision 4
# speedup vs baseline: 1.1197x; 1.1197x over previous
"""AWPINN wavelet-PINN kernel for 8x Trainium2 NeuronCores (Bass/Tile).

Math: for each point i and wavelet k (N=65536, K=512):
  xt = wx*x - bx (same y,z);  s = xt^2+yt^2+zt^2;  E = exp(-0.5*s)
  W  = xt*yt*zt*E          (reference's xw*yw*zw = -W)
  output = sum_k (-coeff*scale)_k * W + bias
  d2u_dx2 = sum_k (coeff*scale*wx^2)_k * (3 - xt^2) * W   (same y,z)

Device structure:
  - s and T3=xt*yt*zt are low-rank bilinear forms in per-point features
    F = [x2,y2,z2,xyz,xy,xz,yz,x,y,z,1] -> TensorEngine matmuls
    (contraction = features, M = 128 wavelets/block, FD = 512 points).
  - All matmuls fp16 (1 cycle/column on PE; fp32/fp32r are ~3x slower).
    Near-fp32 precision via hi/lo splits stacked along the contraction dim:
    [Lh;Ll;Lh] @ [Fh;Fh;Fl] == L @ F with ~2^-21 products. The constant
    feature's lo-row is dropped -> exactly 32 rows, so the four feature
    matmuls of a k-block pair rotate over the PE's four 32-row groups
    (tile_position) and their weight loads overlap the previous matmul.
  - d2 terms decompose via xt^2 = wx^2*x^2 - 2*wx*bx*x + bx^2 into 3
    matvec columns each -> one [128k, 10] output matmul per k-block
    (fp16 hi only; the rel-err budget is ~2e-2, fp16 weights give ~1e-3).
  - Point-chunks are processed in PAIRS sharing one stationary set:
    matmuls are ordered stationary-major so each Ls/Lt quadrant tile is
    loaded once per pair of chunks, halving LDWEIGHTS traffic.
  - The d2 recombination (x^2*R1 + x*R2 + R3 etc.) runs on the HOST:
    the device ships the 10 reduced rows R[10, NP] straight to DRAM,
    eliminating the on-device epilogue + DRAM transpose bounce entirely.
Data parallel over points: each core handles 8192 points; no collectives.
"""

import numpy as np

N_TOTAL = 65536
K_TOTAL = 512
N_CORES = 8
NP_CORE = N_TOTAL // N_CORES        # 8192 points per core
CHUNK = 512                         # points per matmul (PSUM bank = 512 fp32)
N_CHUNKS = NP_CORE // CHUNK         # 16
N_GROUPS = N_CHUNKS // 2            # chunk pairs sharing stationary loads
KBLK = K_TOTAL // 128               # 4 wavelet blocks of 128
NFEAT = 11                          # features per point
NST = 32                            # stacked contraction rows (ones-lo dropped)

_COMPILED = {}


def _split16(a):
    """Split fp32 into fp16 hi + fp16 lo (hi+lo carries ~21 mantissa bits)."""
    a = np.ascontiguousarray(a, np.float32)
    hi = a.astype(np.float16)
    lo = np.float32(a - hi.astype(np.float32)).astype(np.float16)
    return hi, lo


def _stack32(L):
    """[11,n] fp32 coeffs -> [32,n] fp16 stack [Lh; Ll; Lh[:10]]."""
    Lh, Ll = _split16(L)
    return np.concatenate([Lh, Ll, Lh[:NFEAT - 1]], axis=0)


def _build_program():
    import concourse.bacc as bacc
    import concourse.mybir as mybir
    import concourse.tile as tile

    f32 = mybir.dt.float32
    f16 = mybir.dt.float16
    AF = mybir.ActivationFunctionType

    nc = bacc.Bacc("TRN2", target_bir_lowering=False, debug=False)

    # fst: feature stack; replicated on-device at partition offsets 0/32/64/96
    fst_d = nc.dram_tensor("fst", [NST, NP_CORE], f16, kind="ExternalInput")
    # lst: rows 0-31 Ls-stack, 32-63 Lt-stack, 64-95 Ls, 96-127 Lt;
    # columns grouped by k-block
    lst_d = nc.dram_tensor("lst", [128, K_TOTAL], f16, kind="ExternalInput")
    loh_d = nc.dram_tensor("loh", [128, KBLK * 10], f16, kind="ExternalInput")
    out_d = nc.dram_tensor("out", [10, NP_CORE], f32, kind="ExternalOutput")

    with tile.TileContext(nc) as tc:
        with (
            tc.tile_pool(name="persist", bufs=1) as pp,
            tc.tile_pool(name="epool", bufs=4) as ep,
            tc.tile_pool(name="psum_s", bufs=3, space="PSUM") as psps,
            tc.tile_pool(name="psum_t", bufs=3, space="PSUM") as pspt,
            tc.tile_pool(name="psum_out", bufs=2, space="PSUM") as pso,
            tc.tile_pool(name="wpool", bufs=16) as wpool,
        ):
            lst_t = pp.tile([128, K_TOTAL], f16, tag="lst")
            nc.gpsimd.dma_start(lst_t[:], lst_d[:])

            # warm the EXP activation-table load during the initial DMAs so
            # it is off the critical path of the first real exp
            warm = pp.tile([128, 1], f32, tag="warm")
            nc.gpsimd.memset(warm[:], 0.0)
            nc.scalar.activation(warm[:], warm[:], AF.Exp, scale=-0.5)

            # persistent feature stack, replicated across the 4 row groups;
            # loaded in point-eighths so compute starts after the first slice
            f_all = pp.tile([4 * NST, NP_CORE], f16, tag="f_all")
            # first two slices are single chunks so matmuls start sooner
            bounds = [0, CHUNK, 2 * CHUNK] + [
                q * NP_CORE // 8 for q in range(2, 9)]
            for q in range(len(bounds) - 1):
                qs = slice(bounds[q], bounds[q + 1])
                for g in range(4):
                    eng = nc.scalar if (q == 0 and g >= 2) else nc.sync
                    eng.dma_start(f_all[32 * g:32 * (g + 1), qs],
                                  fst_d[:, qs])
                if q == 0:
                    loh_t = pp.tile([128, KBLK * 10], f16, tag="loh")
                    nc.scalar.dma_start(loh_t[:], loh_d[:])

            # reduced rows R, staged in SBUF then DMA'd out per chunk-pair
            r_rows = pp.tile([10, NP_CORE], f32, tag="r_rows")

            pending = []   # (group, w tiles keyed [kb][chunk-in-pair])

            def emit_outs():
                if not pending:
                    return
                t0, w_ts = pending.pop()
                cs = (2 * t0, 2 * t0 + 1)
                po = [pso.tile([10, CHUNK], f32, tag="po", name=f"po{t0}_{j}")
                      for j in range(2)]
                for kb in range(KBLK):
                    for j in range(2):
                        nc.tensor.matmul(
                            po[j][:],
                            loh_t[:, kb * 10:(kb + 1) * 10],
                            w_ts[kb][j][:],
                            start=(kb == 0), stop=(kb == KBLK - 1))
                # drain: one copy on scalar, one on vector, then DMA out
                nc.scalar.copy(
                    r_rows[:, cs[0] * CHUNK:(cs[0] + 1) * CHUNK], po[0][:])
                nc.vector.tensor_copy(
                    r_rows[:, cs[1] * CHUNK:(cs[1] + 1) * CHUNK], po[1][:])
                nc.sync.dma_start(
                    out_d[:, 2 * t0 * CHUNK:(2 * t0 + 2) * CHUNK],
                    r_rows[:, 2 * t0 * CHUNK:(2 * t0 + 2) * CHUNK])

            for t in range(N_GROUPS):
                cA, cB = 2 * t, 2 * t + 1
                fA = f_all[:, cA * CHUNK:(cA + 1) * CHUNK]
                fB = f_all[:, cB * CHUNK:(cB + 1) * CHUNK]
                w_ts = [[None, None] for _ in range(KBLK)]
                for p in range(KBLK // 2):      # k-block pairs
                    for half in range(2):
                        kb = 2 * p + half
                        gs, gt = 2 * half, 2 * half + 1
                        ks = slice(kb * 128, (kb + 1) * 128)
                        # s matmuls for both chunks off one stationary load
                        xs = [psps.tile([128, CHUNK], f32, tag="ps_s",
                                        name=f"s{t}_{kb}_{j}")
                              for j in range(2)]
                        for j, f_t in ((0, fA), (1, fB)):
                            nc.tensor.matmul(
                                xs[j][:],
                                lst_t[32 * gs:32 * (gs + 1), ks],
                                f_t[32 * gs:32 * (gs + 1), :],
                                start=True, stop=True,
                                tile_position=(32 * gs, 0))
                        ys = [pspt.tile([128, CHUNK], f32, tag="ps_t",
                                        name=f"t{t}_{kb}_{j}")
                              for j in range(2)]
                        for j, f_t in ((0, fA), (1, fB)):
                            nc.tensor.matmul(
                                ys[j][:],
                                lst_t[32 * gt:32 * (gt + 1), ks],
                                f_t[32 * gt:32 * (gt + 1), :],
                                start=True, stop=True,
                                tile_position=(32 * gt, 0))
                        for j in range(2):
                            e_t = ep.tile([128, CHUNK], f32, tag="e",
                                          name=f"e{t}_{kb}_{j}")
                            nc.scalar.activation(
                                e_t[:], xs[j][:], AF.Exp, scale=-0.5)
                            w_t = wpool.tile([128, CHUNK], f16, tag="w",
                                             name=f"w{t}_{kb}_{j}")
                            nc.vector.tensor_mul(w_t[:], ys[j][:], e_t[:])
                            w_ts[kb][j] = w_t
                    if p == 0:
                        emit_outs()   # previous group's output matmuls,
                                      # after this group's first stationary
                                      # block has been consumed
                pending.append((t, w_ts))
            emit_outs()
    nc.compile()
    return nc


def _get_program():
    if "nc" not in _COMPILED:
        _COMPILED["nc"] = _build_program()
    return _COMPILED["nc"]


def _host_prep(x, y, z, wx, bx, wy, by, wz, bz, coeff):
    """Build per-core input maps (features + coefficient matrices)."""
    f8 = np.float64
    wx64, bx64 = wx.astype(f8), bx.astype(f8)
    wy64, by64 = wy.astype(f8), by.astype(f8)
    wz64, bz64 = wz.astype(f8), bz.astype(f8)
    c64 = coeff.astype(f8)
    sc = np.sqrt(np.clip(wx64 * wy64 * wz64, 1e-12, None))
    Z = np.zeros_like(wx64)

    # s = xt^2 + yt^2 + zt^2 over features [x2,y2,z2,xyz,xy,xz,yz,x,y,z,1]
    Ls = np.stack([
        wx64 ** 2, wy64 ** 2, wz64 ** 2, Z, Z, Z, Z,
        -2 * wx64 * bx64, -2 * wy64 * by64, -2 * wz64 * bz64,
        bx64 ** 2 + by64 ** 2 + bz64 ** 2,
    ]).astype(np.float32)                      # [11, K]
    # T3 = xt*yt*zt
    Lt = np.stack([
        Z, Z, Z,
        wx64 * wy64 * wz64, -wx64 * wy64 * bz64, -wx64 * by64 * wz64,
        -bx64 * wy64 * wz64, wx64 * by64 * bz64, bx64 * wy64 * bz64,
        bx64 * by64 * wz64, -bx64 * by64 * bz64,
    ]).astype(np.float32)                      # [11, K]
    b1 = c64 * sc * wx64 ** 2
    b2 = c64 * sc * wy64 ** 2
    b3 = c64 * sc * wz64 ** 2
    Lo = np.stack([
        -c64 * sc,
        -b1 * wx64 ** 2, 2 * b1 * wx64 * bx64, b1 * (3 - bx64 ** 2),
        -b2 * wy64 ** 2, 2 * b2 * wy64 * by64, b2 * (3 - by64 ** 2),
        -b3 * wz64 ** 2, 2 * b3 * wz64 * bz64, b3 * (3 - bz64 ** 2),
    ], axis=1).astype(np.float32)              # [K, 10]

    Ls32 = _stack32(Ls)                        # [32, K] fp16
    Lt32 = _stack32(Lt)
    lst_pack = np.concatenate([Ls32, Lt32, Ls32, Lt32], axis=0)  # [128, K]
    Loh = Lo.astype(np.float16)
    loh_pack = np.concatenate(
        [Loh[kb * 128:(kb + 1) * 128] for kb in range(KBLK)], axis=1)  # [128, 40]

    in_maps = []
    for cid in range(N_CORES):
        sl = slice(cid * NP_CORE, (cid + 1) * NP_CORE)
        xs, ys, zs = (np.ascontiguousarray(a[sl], np.float32) for a in (x, y, z))
        F = np.stack([
            xs * xs, ys * ys, zs * zs, xs * ys * zs, xs * ys, xs * zs,
            ys * zs, xs, ys, zs, np.ones_like(xs),
        ]).astype(np.float32)                  # [11, NP_CORE]
        Fh, Fl = _split16(F)
        fst = np.concatenate([Fh, Fh, Fl[:NFEAT - 1]], axis=0)   # [32, NP]
        in_maps.append({"fst": fst, "lst": lst_pack, "loh": loh_pack})
    return in_maps


def _run_device(in_maps, trace=False):
    from concourse.bass_utils import run_bass_kernel_spmd
    nc = _get_program()
    last_err = None
    for _attempt in range(3):
        try:
            return run_bass_kernel_spmd(
                nc, in_maps, list(range(N_CORES)), trace=trace)
        except Exception as ex:  # transient NRT device errors recover on retry
            last_err = ex
    raise last_err


def kernel(x, y, z, wx, bx, wy, by, wz, bz, coeff, bias, _trace=False):
    x, y, z = (np.asarray(a, np.float32) for a in (x, y, z))
    in_maps = _host_prep(
        x, y, z,
        *(np.asarray(a, np.float32) for a in (wx, bx, wy, by, wz, bz, coeff)))
    res = _run_device(in_maps, trace=_trace)
    R = np.concatenate(
        [res.results[cid]["out"] for cid in range(N_CORES)], axis=1)  # [10, N]
    bias_f = np.float32(np.asarray(bias))
    x64, y64, z64 = (a.astype(np.float64) for a in (x, y, z))
    R64 = R.astype(np.float64)
    output = (R64[0] + np.float64(bias_f)).astype(np.float32)
    d2x = (x64 * x64 * R64[1] + x64 * R64[2] + R64[3]).astype(np.float32)
    d2y = (y64 * y64 * R64[4] + y64 * R64[5] + R64[6]).astype(np.float32)
    d2z = (z64 * z64 * R64[7] + z64 * R64[8] + R64[9]).astype(np.float32)
    if _trace:
        kernel._last_results = res
    return (output, d2x, d2y, d2z)
